# revision 15
# baseline (speedup 1.0000x reference)
"""BitNet ternary linear (nn_BitNetLinear4Bit) Trainium2 Bass kernel.

out = x @ (alpha * clip(round(w/alpha), -1, 1))^T + bias
  x: [2, 2048, 4096] f32, w: [11008, 4096] f32, alpha: [1] f32, bias: [11008] f32
  -> out: [2, 2048, 11008] f32

Sharding: column-parallel over 8 cores. Each core gets the full x
(replicated) and a 1376-row slice of w / bias; it produces a
[4096, 1376] slice of the output which the host concatenates.

Per-core algorithm (all math on device):
  Phase W: stream w-shard row-blocks, ternarize (t = sign(w) * (|w| >=
    a/2), exact in bf16; equals clip(round(w/a),-1,1) away from the
    measure-zero half-integer boundary), XBAR-transpose into resident
    SBUF weights wt_c[128, 32, 128] bf16, one tensor per 128-wide
    output-feature chunk (keeps every tensor < 64KB/partition and every
    XBAR destination contiguous).
  Phase MM: per 128-token block: DMA x rows (contiguous f32), cast to
    bf16, XBAR-transpose to xt [128, 32, 128]; accumulate K=4096 with
    bf16 matmuls (N=128 rhs chunks, 3 PSUM banks); evict with
    ACT copy*alpha then DVE +bias; DMA out.

alpha is read on the host and baked into the program as an immediate;
the compiled program is cached keyed on alpha and recompiled if it
changes.
"""

import numpy as np

B, S, DIN, DOUT = 2, 2048, 4096, 11008
NCORES = 8
DOUT_SH = DOUT // NCORES  # 1376
TOK = B * S  # 4096
P = 128


def _build(alpha_f, TOK=TOK, DIN=DIN, DOUT_SH=DOUT_SH, debug=False):
    import concourse.mybir as mybir
    from concourse import bacc
    from concourse.tile import TileContext

    f32 = mybir.dt.float32
    bf16 = mybir.dt.bfloat16
    Alu = mybir.AluOpType
    Act = mybir.ActivationFunctionType

    KO = DIN // P
    M_SUBS = TOK // P
    W_CHUNKS = (DOUT_SH + P - 1) // P  # 11 (last chunk 96 rows)
    QCOL = min(1024, DIN)
    QK = QCOL // P  # ko levels per quantize chunk
    # psum tiles cover groups of 4 dout chunks (<=512 wide)
    PSUM_GROUPS = []
    c = 0
    while c < W_CHUNKS:
        hi = min(c + 4, W_CHUNKS)
        width = min(DOUT_SH, hi * P) - c * P
        PSUM_GROUPS.append((c, hi, c * P, width))
        c = hi

    a2 = float(alpha_f) * 0.5

    nc = bacc.Bacc(None, target_bir_lowering=False, debug=debug)
    x_d = nc.dram_tensor("x", [TOK, DIN], f32, kind="ExternalInput")
    w_d = nc.dram_tensor("w", [DOUT_SH, DIN], f32, kind="ExternalInput")
    nc.dram_tensor("alpha", [1], f32, kind="ExternalInput")
    b_d = nc.dram_tensor("bias", [DOUT_SH], f32, kind="ExternalInput")
    o_d = nc.dram_tensor("out", [TOK, DOUT_SH], f32, kind="ExternalOutput")

    with TileContext(nc) as tc:
        with (
            tc.tile_pool(name="const", bufs=1) as const,
            tc.tile_pool(name="wres", bufs=1) as wres,
        ):
            bias_sb = const.tile([P, DOUT_SH], f32)
            nc.sync.dma_start(
                bias_sb[:],
                b_d[:].rearrange("(a n) -> a n", a=1).to_broadcast((P, DOUT_SH)),
            )

            # resident transposed ternary weights, one tensor per dout chunk:
            # wt[c][p, ko, j] = t[c*128+j, ko*128+p]
            wt = [
                wres.tile(
                    [P, KO, min(P, DOUT_SH - c * P)], bf16, name=f"wt_{c}"
                )
                for c in range(W_CHUNKS)
            ]

            # ---- Phase W: quantize + transpose w shard ----
            with tc.tile_pool(name="wq", bufs=3) as wq:
                for c in range(W_CHUNKS):
                    rc = min(P, DOUT_SH - c * P)  # 128 or 96 (last)
                    for q in range(DIN // QCOL):
                        wrow = wq.tile([P, QCOL], f32, tag="wrow")
                        nc.sync.dma_start(
                            wrow[:rc, :],
                            w_d[c * P : c * P + rc, q * QCOL : (q + 1) * QCOL],
                        )
                        # t = (w >= a/2) - (w <= -a/2) in {-1,0,1}
                        le = wq.tile([P, QCOL], bf16, tag="le")
                        nc.vector.tensor_scalar(
                            le[:rc], wrow[:rc], -a2, None, Alu.is_le
                        )
                        ge = wq.tile([P, QCOL], bf16, tag="ge")
                        nc.vector.tensor_scalar(
                            ge[:rc], wrow[:rc], a2, None, Alu.is_ge
                        )
                        tq = wq.tile([P, QCOL], bf16, tag="tq")
                        nc.vector.tensor_sub(tq[:rc], ge[:rc], le[:rc])
                        # XBAR transpose [rc, QCOL] -> [128, QK, rc]
                        nc.sync.dma_start_transpose(
                            wt[c][:, q * QK : (q + 1) * QK, :], tq[:rc, :]
                        )

            # ---- Phase MM ----
            with (
                tc.tile_pool(name="xp", bufs=2) as xp,
                tc.tile_pool(name="op", bufs=2) as op,
                tc.tile_pool(name="pso", bufs=6, space="PSUM") as pso,
            ):
                for ms in range(M_SUBS):
                    xbf = xp.tile([P, DIN], bf16, tag="xbf")
                    for h in range(2):
                        hw = DIN // 2
                        xrow = xp.tile([P, hw], f32, tag="xrow")
                        nc.sync.dma_start(
                            xrow[:], x_d[ms * P : (ms + 1) * P, h * hw : (h + 1) * hw]
                        )
                        nc.any.tensor_copy(xbf[:, h * hw : (h + 1) * hw], xrow[:])
                    xt = xp.tile([P, KO, P], bf16, tag="xt")
                    nc.sync.dma_start_transpose(xt[:], xbf[:])

                    psums = [
                        pso.tile([P, 512], f32, tag="po", name=f"po_{ms}_{g}")[
                            :, :width
                        ]
                        for g, (_, _, _, width) in enumerate(PSUM_GROUPS)
                    ]
                    for g, (clo, chi, n0, width) in enumerate(PSUM_GROUPS):
                        for c in range(clo, chi):
                            cw = min(P, DOUT_SH - c * P)
                            off = c * P - n0
                            for ko in range(KO):
                                nc.tensor.matmul(
                                    psums[g][:, off : off + cw],
                                    xt[:, ko, :],
                                    wt[c][:, ko, :],
                                    start=(ko == 0),
                                    stop=(ko == KO - 1),
                                )
                    out_sb = op.tile([P, DOUT_SH], f32, tag="osb")
                    for g, (clo, chi, n0, width) in enumerate(PSUM_GROUPS):
                        # out = psum * alpha  (ACT), then += bias (DVE)
                        nc.scalar.activation(
                            out_sb[:, n0 : n0 + width],
                            psums[g][:],
                            Act.Copy,
                            scale=float(alpha_f),
                        )
                        nc.vector.tensor_add(
                            out_sb[:, n0 : n0 + width],
                            out_sb[:, n0 : n0 + width],
                            bias_sb[:, n0 : n0 + width],
                        )
                    nc.sync.dma_start(o_d[ms * P : (ms + 1) * P, :], out_sb[:])

    nc.compile()
    return nc


_CACHE = {}


def _get_nc(alpha_f):
    key = float(alpha_f)
    if key not in _CACHE:
        _CACHE[key] = _build(key)
    return _CACHE[key]


def kernel(x, w, alpha, bias):
    from concourse.bass_utils import run_bass_kernel_spmd

    alpha2 = np.ascontiguousarray(np.asarray(alpha, dtype=np.float32).reshape(1))
    nc = _get_nc(alpha2[0])
    x2 = np.ascontiguousarray(np.asarray(x, dtype=np.float32).reshape(TOK, DIN))
    in_maps = []
    for c in range(NCORES):
        in_maps.append(
            {
                "x": x2,
                "w": np.ascontiguousarray(w[c * DOUT_SH : (c + 1) * DOUT_SH]),
                "alpha": alpha2,
                "bias": np.ascontiguousarray(bias[c * DOUT_SH : (c + 1) * DOUT_SH]),
            }
        )
    res = run_bass_kernel_spmd(nc, in_maps, core_ids=list(range(NCORES)))
    outs = [res.results[c]["out"] for c in range(NCORES)]
    out = np.concatenate(outs, axis=1).reshape(B, S, DOUT)
    return np.ascontiguousarray(out.astype(np.float32))
